# revision 2
# baseline (speedup 1.0000x reference)
import numpy as np
import jax
import jax.numpy as jnp
from functools import partial

# nn_Attention: BatchNorm1d(train) -> per-partition QKV (shared 256x256
# Linears) -> per-example attention over the 16 feature partitions ->
# residual.  Data-parallel over batch across 8 NeuronCores; BN batch
# statistics are all-reduced with lax.psum; params are replicated.

N_CORES = 8
B = 4096
IN_SIZE = 4096
N_PARTITIONS = 16
D = IN_SIZE // N_PARTITIONS  # 256
EPS = 1e-5

_HI = jax.lax.Precision.HIGHEST


@partial(jax.pmap, axis_name="i",
         in_axes=(0, None, None, None, None, None, None, None, None))
def _attn_dp(x, WQ_w, WQ_b, WK_w, WK_b, WV_w, WV_b, bn_gamma, bn_beta):
    Bl = x.shape[0]  # local batch (512)
    # BatchNorm in training mode: global batch statistics via all-reduce.
    s1 = jax.lax.psum(jnp.sum(x, axis=0), "i")
    s2 = jax.lax.psum(jnp.sum(x * x, axis=0), "i")
    mean = s1 / B
    var = s2 / B - mean * mean
    x_norm = (x - mean) * jax.lax.rsqrt(var + EPS) * bn_gamma + bn_beta

    xp = x_norm.reshape(Bl, N_PARTITIONS, D)
    scale = jnp.asarray(np.sqrt(D), dtype=x.dtype)

    def chunk_attn(xpc):
        Q = jnp.einsum("bpd,ed->bep", xpc, WQ_w, precision=_HI) + WQ_b[None, :, None]
        K = jnp.einsum("bpd,ed->bep", xpc, WK_w, precision=_HI) + WK_b[None, :, None]
        V = jnp.einsum("bpd,ed->bep", xpc, WV_w, precision=_HI) + WV_b[None, :, None]
        dot = jnp.einsum("bep,bfp->bef", Q, K, precision=_HI) / scale
        attn = jax.nn.softmax(dot, axis=2)
        return jnp.einsum("bef,bfp->bep", attn, V, precision=_HI)

    CH = 64
    prod = jax.lax.map(chunk_attn, xp.reshape(Bl // CH, CH, N_PARTITIONS, D))
    return prod.reshape(Bl, D * N_PARTITIONS) + x


def kernel(**inputs):
    x = np.ascontiguousarray(inputs["x"], dtype=np.float32)
    xs = x.reshape(N_CORES, B // N_CORES, IN_SIZE)
    args = [np.asarray(inputs[k], dtype=np.float32) for k in
            ("WQ_w", "WQ_b", "WK_w", "WK_b", "WV_w", "WV_b",
             "bn_gamma", "bn_beta")]
    out = _attn_dp(xs, *args)
    return np.asarray(out).reshape(B, IN_SIZE).astype(np.float32)


# revision 4
# speedup vs baseline: 1.1818x; 1.1818x over previous
import numpy as np
import jax
import jax.numpy as jnp
from functools import partial

# nn_Attention: BatchNorm1d(train) -> per-partition QKV (shared 256x256
# Linears) -> per-example attention over the 16 feature partitions ->
# residual.  Data-parallel over batch across 8 NeuronCores; BN batch
# statistics are all-reduced with lax.psum; params are replicated.

N_CORES = 8
B = 4096
IN_SIZE = 4096
N_PARTITIONS = 16
D = IN_SIZE // N_PARTITIONS  # 256
EPS = 1e-5

_HI = jax.lax.Precision.HIGHEST


@partial(jax.pmap, axis_name="i",
         in_axes=(0, None, None, None, None, None, None, None, None))
def _attn_dp(x, WQ_w, WQ_b, WK_w, WK_b, WV_w, WV_b, bn_gamma, bn_beta):
    Bl = x.shape[0]  # local batch (512)
    # BatchNorm in training mode: global batch statistics via all-reduce.
    s1 = jax.lax.psum(jnp.sum(x, axis=0), "i")
    s2 = jax.lax.psum(jnp.sum(x * x, axis=0), "i")
    mean = s1 / B
    var = s2 / B - mean * mean
    x_norm = (x - mean) * jax.lax.rsqrt(var + EPS) * bn_gamma + bn_beta

    xp = x_norm.reshape(Bl, N_PARTITIONS, D)
    scale = jnp.asarray(np.sqrt(D), dtype=x.dtype)

    def chunk_attn(xpc):
        Q = jnp.einsum("bpd,ed->bep", xpc, WQ_w, precision=_HI) + WQ_b[None, :, None]
        K = jnp.einsum("bpd,ed->bep", xpc, WK_w, precision=_HI) + WK_b[None, :, None]
        V = jnp.einsum("bpd,ed->bep", xpc, WV_w, precision=_HI) + WV_b[None, :, None]
        dot = jnp.einsum("bep,bfp->bef", Q, K, precision=_HI) / scale
        attn = jax.nn.softmax(dot, axis=2)
        return jnp.einsum("bef,bfp->bep", attn, V, precision=_HI)

    CH = 64
    prod = jax.lax.map(chunk_attn, xp.reshape(Bl // CH, CH, N_PARTITIONS, D))
    return prod.reshape(Bl, D * N_PARTITIONS) + x


def kernel(**inputs):
    x = np.ascontiguousarray(inputs["x"], dtype=np.float32)
    xs = x.reshape(N_CORES, B // N_CORES, IN_SIZE)
    args = [np.asarray(inputs[k], dtype=np.float32) for k in
            ("WQ_w", "WQ_b", "WK_w", "WK_b", "WV_w", "WV_b",
             "bn_gamma", "bn_beta")]
    out = _attn_dp(xs, *args)
    return np.asarray(out).reshape(B, IN_SIZE).astype(np.float32)
